# revision 4
# baseline (speedup 1.0000x reference)
"""AttentionFlowLayer (BiDAF-style) Trainium2 kernel, 8-core data-parallel.

Reference math per batch n:
  sim[p,a] = H[n,p]@wc + U[n,a]@wa + (H[n,p]*wp)@U[n,a]      (P x A)
  att      = softmax_a(sim)
  U_t      = att @ U[n]                                       (P x 2d)
  b[p]     = max_a sim[p,a]
  H_row    = b @ H[n]                                         (2d,)
  G        = [H | U_t | H*U_t | H*H_row]                      (P x 8d)

Sharding: data-parallel over batch N=16 -> 2 batches per core, no collectives.

Kernel structure per batch (16 p-tiles of 128 rows):
  sim tile S[p,a] computed on PE as (H^T chunks).T @ [wp*U^T | wc] in bf16
  (fp32 PSUM accumulate), + Ua broadcast via a K=1 ones-row matmul.
  Softmax stats on DVE; exp on ACT with accum_out as the denominator;
  e transposed on PE for the e@U matmul; b@H row + ones outer-product
  broadcast for H_row.
"""

import sys
import types

import numpy as np

N, P, A, D2 = 16, 2048, 256, 256
NCORES = 8
NPC = N // NCORES  # batches per core
PT = P // 128  # p-tiles per batch
D8 = 4 * D2  # output feature dim (8*hidden)

_CACHE = {}


def _split_waits(nc, mybir, max_waits=1):
    """The pinned walrus rejects instructions carrying more than one or two
    semaphore waits ("Too many sync wait commands"). Hoist excess waits into
    dedicated EventSemaphore instructions emitted just before the owner on
    the same engine — semantically identical (the sequencer blocks on the
    same conditions at the same program point)."""
    ctr = 0
    for fn in nc.m.functions:
        for bb in fn.blocks:
            il = bb.instructions
            out = []
            for ins in il:
                si = ins.sync_info
                waits = list(si.on_wait) if si is not None and si.on_wait else []
                if len(waits) > max_waits:
                    for w in waits[:-max_waits]:
                        evs = mybir.InstEventSemaphore(
                            name=f"wsplit{ctr}_{ins.name}",
                            engine=ins.engine,
                            sync_info=mybir.SyncInfo(on_wait=[w], on_update=[]),
                        )
                        ctr += 1
                        out.append(evs)
                    si.on_wait = waits[-max_waits:]
                out.append(ins)
            bb.instructions = out


def _build():
    import concourse.bass as bass
    import concourse.mybir as mybir
    from concourse.masks import make_identity

    dt = mybir.dt
    from concourse.tile import TileContext

    nc = bass.Bass("TRN2", target_bir_lowering=False, debug=False, num_devices=NCORES)
    h_dram = nc.declare_dram_parameter("H", [NPC, P, D2], dt.float32, isOutput=False)
    u_dram = nc.declare_dram_parameter("U", [NPC, A, D2], dt.float32, isOutput=False)
    w_dram = nc.declare_dram_parameter("W", [1, 3 * D2], dt.float32, isOutput=False)
    g_dram = nc.declare_dram_parameter("G", [NPC, P, D8], dt.float32, isOutput=True)

    with TileContext(nc) as tc:
        with (
            tc.tile_pool(name="const", bufs=1) as cpool,
            tc.tile_pool(name="pern", bufs=1) as npool,
            tc.tile_pool(name="stream", bufs=3) as spool,
            tc.tile_pool(name="small", bufs=4) as smpool,
            tc.tile_pool(name="ps_s", bufs=2, space="PSUM") as ps_s,
            tc.tile_pool(name="ps_t", bufs=2, space="PSUM") as ps_t,
            tc.tile_pool(name="ps_tp", bufs=2, space="PSUM") as ps_tp,
            tc.tile_pool(name="ps_misc", bufs=2, space="PSUM") as ps_misc,
        ):
            ident = cpool.tile([128, 128], dt.bfloat16, tag="ident")
            make_identity(nc, ident)
            ones_bf = cpool.tile([1, 128], dt.bfloat16, tag="ones_bf")
            nc.vector.memset(ones_bf, 1.0)
            ones_f32 = cpool.tile([1, 128], dt.float32, tag="ones_f32")
            nc.vector.memset(ones_f32, 1.0)
            # w columns: [128, 6] where col s = w[s*128:(s+1)*128]
            # wc = cols 0,1 ; wa = cols 2,3 ; wp = cols 4,5
            w_sb = cpool.tile([128, 6], dt.float32, tag="w_sb")
            nc.sync.dma_start(w_sb, w_dram[0, :].rearrange("(s p) -> p s", p=128))
            wa_bf = cpool.tile([128, 2], dt.bfloat16, tag="wa_bf")
            nc.vector.tensor_copy(wa_bf, w_sb[:, 2:4])

            for n in range(NPC):
                # ---- preload U for this batch ----
                u_bf = []  # a-tile chunks [128a, 256d] bf16 (mm2 rhs)
                for a_t in range(2):
                    u_f = spool.tile([128, D2], dt.float32, tag="u_f")
                    nc.sync.dma_start(u_f, u_dram[n, a_t * 128 : (a_t + 1) * 128, :])
                    ub = npool.tile([128, D2], dt.bfloat16, tag=f"u_bf{n}_{a_t}")
                    nc.gpsimd.tensor_copy(ub, u_f)
                    u_bf.append(ub)
                # U^T per d-tile, then rhs1 = [wp*U^T | wc] and pure U^T
                rhs1 = []
                uT = []
                for d_t in range(2):
                    pt = ps_tp.tile([128, D2], dt.bfloat16, tag="tp")
                    for a_t in range(2):
                        nc.tensor.transpose(
                            pt[:, a_t * 128 : (a_t + 1) * 128],
                            u_bf[a_t][:, d_t * 128 : (d_t + 1) * 128],
                            ident,
                        )
                    r1 = npool.tile([128, D2 + 1], dt.bfloat16, tag=f"rhs1_{n}_{d_t}")
                    nc.vector.tensor_scalar_mul(
                        r1[:, 0:D2], pt, w_sb[:, 4 + d_t : 5 + d_t]
                    )
                    nc.vector.tensor_copy(r1[:, D2 : D2 + 1], w_sb[:, d_t : d_t + 1])
                    rhs1.append(r1)
                    ut = npool.tile([128, D2], dt.bfloat16, tag=f"uT_{n}_{d_t}")
                    nc.vector.tensor_copy(ut, pt)
                    uT.append(ut)
                # ua row = wa @ U^T  -> [1, 257] bf16 (col 256 stays 0)
                ps_ua = ps_misc.tile([128, D2], dt.float32, tag="misc")
                for d_t in range(2):
                    nc.tensor.matmul(
                        ps_ua[:1, :],
                        wa_bf[:, d_t : d_t + 1],
                        uT[d_t],
                        start=(d_t == 0),
                        stop=(d_t == 1),
                    )
                ua_bf = npool.tile([1, D2 + 1], dt.bfloat16, tag=f"ua_{n}")
                nc.vector.memset(ua_bf, 0.0)
                nc.vector.tensor_copy(ua_bf[:1, 0:D2], ps_ua[:1, :])

                b_sb = npool.tile([128, PT], dt.bfloat16, tag=f"b_{n}")
                h_f32 = []
                h_bf16 = []

                # ---- pass 1: per p-tile ----
                for i in range(PT):
                    rows = slice(i * 128, (i + 1) * 128)
                    h = npool.tile([128, D2], dt.float32, tag=f"h{n}_{i}")
                    nc.sync.dma_start(h, h_dram[n, rows, :])
                    hb = npool.tile([128, D2], dt.bfloat16, tag=f"hb{n}_{i}")
                    nc.gpsimd.tensor_copy(hb, h)
                    h_f32.append(h)
                    h_bf16.append(hb)

                    # H^T chunks: [128d, 128p] per d-tile, packed side by side
                    pth = ps_tp.tile([128, D2], dt.bfloat16, tag="tp")
                    for d_t in range(2):
                        nc.tensor.transpose(
                            pth[:, d_t * 128 : (d_t + 1) * 128],
                            hb[:, d_t * 128 : (d_t + 1) * 128],
                            ident,
                        )
                    hT = spool.tile([128, D2], dt.bfloat16, tag="hT")
                    nc.vector.tensor_copy(hT, pth)

                    # sim tile: S[:, 0:256] = sim, S[:, 256] = Hc
                    S = ps_s.tile([128, D2 + 1], dt.float32, tag="S")
                    nc.tensor.matmul(
                        S, hT[:, 0:128], rhs1[0], start=True, stop=False
                    )
                    nc.tensor.matmul(
                        S, hT[:, 128:256], rhs1[1], start=False, stop=False
                    )
                    nc.tensor.matmul(
                        S, ones_bf, ua_bf, start=False, stop=True
                    )

                    m = smpool.tile([128, 1], dt.float32, tag="m")
                    nc.vector.tensor_reduce(
                        m, S[:, 0:D2], axis=mybir.AxisListType.X, op=mybir.AluOpType.max
                    )
                    # b = m + Hc
                    nc.vector.tensor_add(b_sb[:, i : i + 1], m, S[:, D2 : D2 + 1])

                    # e = exp(sim) (values are small; no max subtraction needed),
                    # den = rowsum(e)
                    e_bf = spool.tile([128, D2], dt.bfloat16, tag="e")
                    den = smpool.tile([128, 1], dt.float32, tag="den")
                    nc.scalar.activation(
                        e_bf,
                        S[:, 0:D2],
                        mybir.ActivationFunctionType.Exp,
                        bias=0.0,
                        scale=1.0,
                        accum_out=den,
                    )

                    pte = ps_tp.tile([128, D2], dt.bfloat16, tag="tp")
                    for a_t in range(2):
                        nc.tensor.transpose(
                            pte[:, a_t * 128 : (a_t + 1) * 128],
                            e_bf[:, a_t * 128 : (a_t + 1) * 128],
                            ident,
                        )
                    eT = spool.tile([128, D2], dt.bfloat16, tag="eT")
                    nc.scalar.copy(eT, pte)

                    T = ps_t.tile([128, D2], dt.float32, tag="T")
                    nc.tensor.matmul(T, eT[:, 0:128], u_bf[0], start=True, stop=False)
                    nc.tensor.matmul(
                        T, eT[:, 128:256], u_bf[1], start=False, stop=True
                    )

                    r = smpool.tile([128, 1], dt.float32, tag="r")
                    nc.vector.reciprocal(r, den)
                    g_mid = spool.tile([128, 2 * D2], dt.float32, tag="g_mid")
                    nc.vector.tensor_scalar_mul(g_mid[:, 0:D2], T, r)
                    nc.vector.tensor_mul(g_mid[:, D2 : 2 * D2], h, g_mid[:, 0:D2])

                    nc.sync.dma_start(g_dram[n, rows, 0:D2], h)
                    nc.sync.dma_start(g_dram[n, rows, D2 : 3 * D2], g_mid)

                # ---- pass 2: H_row = b @ H, broadcast, G3 ----
                ps_row = ps_misc.tile([128, D2], dt.float32, tag="misc")
                for i in range(PT):
                    nc.tensor.matmul(
                        ps_row[:1, :],
                        b_sb[:, i : i + 1],
                        h_bf16[i],
                        start=(i == 0),
                        stop=(i == PT - 1),
                    )
                hrow = smpool.tile([1, D2], dt.float32, tag="hrow")
                nc.scalar.copy(hrow, ps_row[:1, :])
                ps_bc = ps_misc.tile([128, D2], dt.float32, tag="misc")
                nc.tensor.matmul(ps_bc, ones_f32, hrow, start=True, stop=True)
                hbc = npool.tile([128, D2], dt.float32, tag=f"hbc{n}")
                nc.scalar.copy(hbc, ps_bc)
                for i in range(PT):
                    rows = slice(i * 128, (i + 1) * 128)
                    g3 = spool.tile([128, D2], dt.float32, tag="g3")
                    nc.gpsimd.tensor_mul(g3, h_f32[i], hbc)
                    nc.sync.dma_start(g_dram[n, rows, 3 * D2 : 4 * D2], g3)

    _split_waits(nc, mybir, max_waits=1)
    return nc


def kernel(H, U, W):
    from concourse.bass_utils import run_bass_kernel_spmd

    if "nc" not in _CACHE:
        _CACHE["nc"] = _build()
    nc = _CACHE["nc"]

    H = np.ascontiguousarray(H, dtype=np.float32)
    U = np.ascontiguousarray(U, dtype=np.float32)
    W = np.ascontiguousarray(W, dtype=np.float32)
    in_maps = [
        {
            "H": H[c * NPC : (c + 1) * NPC],
            "U": U[c * NPC : (c + 1) * NPC],
            "W": W,
        }
        for c in range(NCORES)
    ]
    res = run_bass_kernel_spmd(nc, in_maps, core_ids=list(range(NCORES)))
    return np.concatenate([res.results[c]["G"] for c in range(NCORES)], axis=0)
